# revision 9
# baseline (speedup 1.0000x reference)
"""Trainium2 kernel for nn_BS_Registers_density: out = U @ rho @ U.T.

U = cos(a)*cos_mask + sin(a)*sin_mask + id_mask is the identity outside its
top-left 64x64 corner (32 disjoint 2x2 Givens blocks), so the product only
modifies the first 64 rows and first 64 columns of rho:

  out[0:64,  64:] = B @ rho[0:64, 64:]          (row strip)
  out[64:,  0:64] = rho[64:, 0:64] @ B^T        (col strip)
  out[0:64, 0:64] = B @ rho[0:64, 0:64] @ B^T   (corner)
  out[64:,  64:]  = rho[64:, 64:]               (identity pass-through)

with B = U[0:64, 0:64].  Sharding (per the hint: "the sparse rotation
structure allows replicating only the k affected rows"): each core receives
ONLY the affected data — a 504-column slice of the k=64 affected rows plus a
504-row slice of the affected column strip.  The device computes every
changed output element; the identity pass-through block never transits the
device — the host unshard step pastes the device-computed strips into a copy
of rho (data movement only, no host arithmetic; theta -> sin/cos and all
products happen on device).

Both strip updates are the same 2x2 butterfly on adjacent lanes l=2k,2k+1:

  out[2k] = sin*in[2k] + cos*in[2k+1],  out[2k+1] = -cos*in[2k] + sin*in[2k+1]

row strip: positions = columns, lanes = the 64 affected rows (host packs
transposed); col strip: positions = rows, lanes = the 64 affected columns
(natural layout).  The host splits lanes into even/odd planes so the whole
update is 4 contiguous DVE ops on [128, 256] tiles — no PE serialization,
and exact fp32 2-term arithmetic.  Only the 64x64 corner (needs both sides)
runs on the PE (2 matmuls via the lhsT-transpose trick), concurrently with
the DVE work; cores 1-7 compute a dummy corner the host ignores.

Hardware constraints that shape the code (measured in prior sessions):
  - every instruction encodes at most ONE semaphore wait, so each engine
    instruction depends on at most one cross-engine semaphore (ACT results
    are staged through a DVE copy; an absorber matmul lets the PE observe
    the masks-DMA lane once);
  - the kernel-tail Drain cannot carry one wait per live semaphore, so the
    patched tail below spreads them across SP no-ops;
  - only 2 HWDGE rings exist (SP + ACT); loads and stores are split across
    both, 5 DMAs total (8 completion-sem lanes available).
"""

import numpy as np

N_CORES = 8
N_FULL = 4096
K = 64  # size of the affected corner block
RW = (N_FULL - K) // N_CORES  # 504: strip positions per core per strip
NPOS = 2 * RW  # 1008 butterfly positions per core (+16 pad -> 1024)
NG = 8  # position groups of 128
HW = NG * (K // 2)  # 256: even (or odd) lane-plane width

# masks tensor A layout (f32, [128, AW]):
#   cols    0:64   real cos mask   (partitions 0:64; zeros elsewhere)
#   cols   64:128  real sin mask
#   cols  128:192  real id mask
#   cols  192:256  corner_in = rho[0:64, 0:64]   (core 0; zeros on cores 1-7)
# theta travels in its own 1KB tensor th [128, 2] = (theta, theta+pi/2) so
# the trig -> coefficient chain starts as soon as the ring delivers one
# packet, concurrent with the big data load.
AW = 192 + K  # 256

# data tensor D layout (f32, [128, 512]): cols 0:256 even lanes, 256:512 odd.
# position p = g*128 + partition (g = 0..7): p < 504 -> row strip (transposed
# col slice), 504 <= p < 1008 -> col strip (natural row slice), rest pad.
DW = 2 * HW  # 512

_CACHE = {}


def _patched_drain_and_barrier(self, tick_clock, wait_clock):
    """Kernel-tail replacement for TileContext._drain_and_barrier.

    The stock tail attaches every outstanding semaphore wait to one Drain
    instruction, but the TRN2 instruction encoding holds a single semaphore
    wait, so walrus rejects it ("Too many sync wait commands").  Spread the
    waits across one SP no-op per semaphore instead, then drain + barrier.
    """
    import re

    import bass_rust
    from concourse.vector_clock import ScopedClock

    nc = self.nc
    vals = [int(x) for x in re.findall(r"\d+", repr(tick_clock.global_clock))]
    for proc, val in enumerate(vals):
        if val <= 0:
            continue
        nop = nc.sync.nop()
        mask = bass_rust.VectorClock()
        mask.require_at_least(proc, val)
        wait_clock.add_sem_waits(nop.ins, ScopedClock({None: mask}))

    nc.sync.drain()
    nc.all_engine_barrier()
    popped = nc._tile_sem_poison_stack.pop()
    assert popped is self._sem_poison
    nc.clear_and_free_semaphores(list(self.sems.allocated().values()))
    nc.all_engine_barrier()


def _build_nc():
    import concourse.bass as bass
    import concourse.tile as tile
    from concourse import mybir

    f32 = mybir.dt.float32
    Alu = mybir.AluOpType
    Act = mybir.ActivationFunctionType

    nc = bass.Bass()
    th = nc.dram_tensor("th", [128, 2], f32, kind="ExternalInput")
    masks = nc.dram_tensor("masks", [128, AW], f32, kind="ExternalInput")
    data = nc.dram_tensor("data", [128, DW], f32, kind="ExternalInput")
    outs = nc.dram_tensor("outs", [128, DW], f32, kind="ExternalOutput")
    outc = nc.dram_tensor("outc", [K, K], f32, kind="ExternalOutput")

    tile.TileContext._drain_and_barrier = _patched_drain_and_barrier
    with tile.TileContext(nc) as tc:
        with (
            tc.tile_pool(name="const", bufs=1) as cp,
            tc.tile_pool(name="work", bufs=1) as wp,
            tc.tile_pool(name="ps", bufs=1, space=bass.MemorySpace.PSUM) as ps,
        ):
            # Loads: butterfly data first on the SP ring (it spins up ~1us
            # earlier than the ACT ring); theta then masks on the ACT ring.
            dt = cp.tile([128, DW], f32, tag="dt")
            nc.sync.dma_start(out=dt[:], in_=data[:])
            tht = cp.tile([128, 2], f32, tag="tht")
            nc.scalar.dma_start(out=tht[:], in_=th[:])
            at = cp.tile([128, AW], f32, tag="at")
            nc.scalar.dma_start(out=at[:], in_=masks[:])

            # Absorber: one tiny matmul whose only wait is the masks-DMA
            # lane; after it the PE has observed that lane, so the real
            # matmuls can read `at` directly with just their DVE wait.
            pa = ps.tile([K, K], f32, tag="abs")
            nc.tensor.matmul(pa[:], at[0:K, 0:K], at[0:K, 0:K], start=True, stop=True)

            # s = sin(a); cos(a) = sin(a+pi/2); -cos(a) = sin(-(a+pi/2))
            acts = cp.tile([128, 3], f32, tag="acts")
            nc.scalar.activation(acts[:, 0:1], tht[:, 0:1], Act.Sin)
            nc.scalar.activation(acts[:, 1:2], tht[:, 1:2], Act.Sin)
            nc.scalar.activation(acts[:, 2:3], tht[:, 1:2], Act.Sin, scale=-1.0)
            sc = cp.tile([128, 3], f32, tag="sc")
            nc.vector.tensor_copy(sc[:], acts[:])

            # DVE copy of the mask head absorbs the masks-DMA wait for DVE.
            mk = cp.tile([K, 192], f32, tag="mk")
            nc.vector.tensor_copy(mk[:], at[0:K, 0:192])

            # n = B^T = sin(a)*sinm - cos(a)*cosm + idm  (cosm antisymmetric)
            tmp = cp.tile([K, K], f32, tag="tmp")
            nc.vector.scalar_tensor_tensor(tmp[:], mk[:, 64:128], sc[0:K, 0:1], mk[:, 128:192], Alu.mult, Alu.add)
            n = cp.tile([K, K], f32, tag="n")
            nc.vector.scalar_tensor_tensor(n[:], mk[:, 0:64], sc[0:K, 2:3], tmp[:], Alu.mult, Alu.add)

            # Corner (PE, concurrent with the DVE butterfly):
            # Y = corner_in^T @ B^T = (B @ corner)^T; outc = B @ Y = corner'^T
            py = ps.tile([K, K], f32, tag="y")
            nc.tensor.matmul(py[:], at[0:K, 192:AW], n[:], start=True, stop=True)

            # Butterfly: outE = s*E + c*O ; outO = s*O - c*E
            osb = wp.tile([128, DW], f32, tag="osb")
            q1 = wp.tile([128, HW], f32, tag="q1")
            nc.vector.tensor_scalar_mul(q1[:], dt[:, 0:HW], sc[:, 0:1])
            nc.vector.scalar_tensor_tensor(osb[:, 0:HW], dt[:, HW:DW], sc[:, 1:2], q1[:], Alu.mult, Alu.add)
            q3 = wp.tile([128, HW], f32, tag="q3")
            nc.vector.tensor_scalar_mul(q3[:], dt[:, 0:HW], sc[:, 2:3])
            nc.vector.scalar_tensor_tensor(osb[:, HW:DW], dt[:, HW:DW], sc[:, 0:1], q3[:], Alu.mult, Alu.add)

            # Stores: even plane on the ACT ring, odd plane on the SP ring.
            nc.scalar.dma_start(out=outs[:, 0:HW], in_=osb[:, 0:HW])
            nc.sync.dma_start(out=outs[:, HW:DW], in_=osb[:, HW:DW])

            # Corner tail: PSUM staging + second matmul + store (ACT ring).
            ysb = cp.tile([K, K], f32, tag="ysb")
            nc.vector.tensor_copy(ysb[:], py[:])
            pc0 = ps.tile([K, K], f32, tag="pc0")
            nc.tensor.matmul(pc0[:], n[:], ysb[:], start=True, stop=True)
            oc = cp.tile([K, K], f32, tag="oc")
            nc.vector.tensor_copy(oc[:], pc0[:])
            nc.scalar.dma_start(out=outc[:], in_=oc[:])

    return nc


def _get_nc():
    if "nc" not in _CACHE:
        _CACHE["nc"] = _build_nc()
    return _CACHE["nc"]


def _in_maps(input_state, angle, cos_matrix, sin_matrix, id_matrix):
    rho = np.ascontiguousarray(np.asarray(input_state, dtype=np.float32))
    assert rho.shape == (N_FULL, N_FULL)
    theta = np.float32(np.asarray(angle))

    corner = lambda m: np.asarray(m, dtype=np.float32)[0:K, 0:K]
    am = np.zeros((128, AW), dtype=np.float32)
    am[0:K, 0:64] = corner(cos_matrix)
    am[0:K, 64:128] = corner(sin_matrix)
    am[0:K, 128:192] = corner(id_matrix)
    tht = np.empty((128, 2), dtype=np.float32)
    tht[:, 0] = theta
    tht[:, 1] = theta + np.float32(np.pi / 2)

    maps = []
    for c in range(N_CORES):
        a = am if c else am.copy()
        if c == 0:
            a[0:K, 192:AW] = rho[0:K, 0:K]
        pos = np.zeros((NG * 128, K), dtype=np.float32)
        pos[0:RW] = rho[0:K, K + c * RW : K + (c + 1) * RW].T
        pos[RW:NPOS] = rho[K + c * RW : K + (c + 1) * RW, 0:K]
        # [1024, 64] -> per-group packing [128, 8*32] for even/odd planes
        ev = pos[:, 0::2].reshape(NG, 128, K // 2).transpose(1, 0, 2).reshape(128, HW)
        od = pos[:, 1::2].reshape(NG, 128, K // 2).transpose(1, 0, 2).reshape(128, HW)
        d = np.empty((128, DW), dtype=np.float32)
        d[:, 0:HW] = ev
        d[:, HW:DW] = od
        maps.append({"th": tht, "masks": a, "data": d})
    return maps


def _assemble(input_state, results):
    full = np.array(np.asarray(input_state, dtype=np.float32), copy=True)
    vals = np.empty((NG * 128, K), dtype=np.float32)
    for c in range(N_CORES):
        o = results[c]["outs"]
        vals[:, 0::2] = o[:, 0:HW].reshape(128, NG, K // 2).transpose(1, 0, 2).reshape(NG * 128, K // 2)
        vals[:, 1::2] = o[:, HW:DW].reshape(128, NG, K // 2).transpose(1, 0, 2).reshape(NG * 128, K // 2)
        full[0:K, K + c * RW : K + (c + 1) * RW] = vals[0:RW].T
        full[K + c * RW : K + (c + 1) * RW, 0:K] = vals[RW:NPOS]
    full[0:K, 0:K] = results[0]["outc"].T
    return full


def _patch_walrus_sem_count():
    """Cap the semaphore count walrus declares in the NEFF.

    The NEFF pre/postamble zeroes every declared semaphore one instruction
    at a time (~27ns each, ~6.8us for all 256); this program uses sems
    below ~168, so capping the declared count shrinks that fixed sweep.
    Safe to skip if the bass_utils internals ever change shape.
    """
    if _CACHE.get("walrus_patched"):
        return
    _CACHE["walrus_patched"] = True
    try:
        import concourse.bass_utils as bu

        orig = bu.get_walrus_args
        bu.get_walrus_args = lambda *a, **k: orig(*a, **k) + ["--max-sem-num=168"]
    except Exception:
        pass


def run(input_state, angle, cos_matrix, sin_matrix, id_matrix, **spmd_kwargs):
    from concourse.bass_utils import run_bass_kernel_spmd

    _patch_walrus_sem_count()
    nc = _get_nc()
    maps = _in_maps(input_state, angle, cos_matrix, sin_matrix, id_matrix)
    res = run_bass_kernel_spmd(nc, maps, list(range(N_CORES)), **spmd_kwargs)
    return _assemble(input_state, res.results).astype(np.float32, copy=False), res


def kernel(input_state, angle, cos_matrix, sin_matrix, id_matrix):
    full, _ = run(input_state, angle, cos_matrix, sin_matrix, id_matrix)
    return full


# revision 11
# speedup vs baseline: 1.0501x; 1.0501x over previous
"""Trainium2 kernel for nn_BS_Registers_density: out = U @ rho @ U.T.

U = cos(a)*cos_mask + sin(a)*sin_mask + id_mask is the identity outside its
top-left 64x64 corner (32 disjoint 2x2 Givens blocks), so the product only
modifies the first 64 rows and first 64 columns of rho:

  out[0:64,  64:] = B @ rho[0:64, 64:]          (row strip)
  out[64:,  0:64] = rho[64:, 0:64] @ B^T        (col strip)
  out[0:64, 0:64] = B @ rho[0:64, 0:64] @ B^T   (corner)
  out[64:,  64:]  = rho[64:, 64:]               (identity pass-through)

with B = U[0:64, 0:64].  Sharding (per the hint: "the sparse rotation
structure allows replicating only the k affected rows"): each core receives
ONLY the affected data — a 504-column slice of the k=64 affected rows plus a
504-row slice of the affected column strip.  The device computes every
changed output element; the identity pass-through block never transits the
device — the host unshard step pastes the device-computed strips into a copy
of rho (data movement only, no host arithmetic; theta -> sin/cos and all
products happen on device).

Both strip updates are the same 2x2 butterfly on adjacent lanes l=2k,2k+1:

  out[2k] = sin*in[2k] + cos*in[2k+1],  out[2k+1] = -cos*in[2k] + sin*in[2k+1]

row strip: positions = columns, lanes = the 64 affected rows (host packs
transposed); col strip: positions = rows, lanes = the 64 affected columns
(natural layout).  The host splits lanes into even/odd planes so the whole
update is 4 contiguous DVE ops on [128, 256] tiles — no PE serialization,
and exact fp32 2-term arithmetic.  The 64x64 corner (needs the butterfly on
both sides) runs concurrently: two PE matmuls via the lhsT-transpose trick
(the second on a host-packed column-pair-swapped corner, which turns the
partition-pair mix into an elementwise DVE combine of the two PSUM tiles);
cores 1-7 compute a zero corner the host ignores.

Latency details (measured in prior sessions):
  - theta rides a [2, 130] tensor (theta, theta+pi/2, a row of ones) — a
    [128, 2] tensor would be 128 8-byte DMA descriptors (~3us); instead one
    tiny PE matmul (ones x theta) replicates it across partitions;
  - every instruction encodes at most ONE semaphore wait (ACT results are
    staged through a DVE copy; an absorber matmul lets the PE observe the
    masks-DMA lane once);
  - the kernel-tail Drain cannot carry one wait per live semaphore, so the
    patched tail below spreads them across SP no-ops;
  - only 2 HWDGE rings exist (SP + ACT); the SP ring spins up ~1us earlier,
    so the butterfly data loads there; 6 DMAs total (8 lanes available).
"""

import numpy as np

N_CORES = 8
N_FULL = 4096
K = 64  # size of the affected corner block
RW = (N_FULL - K) // N_CORES  # 504: strip positions per core per strip
NPOS = 2 * RW  # 1008 butterfly positions per core (+16 pad -> 1024)
NG = 8  # position groups of 128
HW = NG * (K // 2)  # 256: even (or odd) lane-plane width

# masks tensor layout (f32, [128, AW]), constants on partitions 0:64:
#   cols    0:64   real cos mask
#   cols   64:128  real sin mask
#   cols  128:192  real id mask
#   cols  192:256  corner_in  = rho[0:64, 0:64]        (core 0 only)
#   cols  256:320  corner_sw  = corner_in with column pairs swapped
#   col   320      parity column (+1 even partition, -1 odd)
AW = 321

# th tensor (f32, [2, 130]): row 0 = (theta, theta+pi/2, ones[128]), row 1 = 0
# data tensor D (f32, [128, 512]): cols 0:256 even lanes, 256:512 odd.
# position p = g*128 + partition (g = 0..7): p < 504 -> row strip (transposed
# col slice), 504 <= p < 1008 -> col strip (natural row slice), rest pad.
DW = 2 * HW  # 512

_CACHE = {}


def _patched_drain_and_barrier(self, tick_clock, wait_clock):
    """Kernel-tail replacement for TileContext._drain_and_barrier.

    The stock tail attaches every outstanding semaphore wait to one Drain
    instruction, but the TRN2 instruction encoding holds a single semaphore
    wait, so walrus rejects it ("Too many sync wait commands").  Spread the
    waits across one SP no-op per semaphore instead, then drain + barrier.
    """
    import re

    import bass_rust
    from concourse.vector_clock import ScopedClock

    nc = self.nc
    vals = [int(x) for x in re.findall(r"\d+", repr(tick_clock.global_clock))]
    for proc, val in enumerate(vals):
        if val <= 0:
            continue
        nop = nc.sync.nop()
        mask = bass_rust.VectorClock()
        mask.require_at_least(proc, val)
        wait_clock.add_sem_waits(nop.ins, ScopedClock({None: mask}))

    nc.sync.drain()
    nc.all_engine_barrier()
    popped = nc._tile_sem_poison_stack.pop()
    assert popped is self._sem_poison
    nc.clear_and_free_semaphores(list(self.sems.allocated().values()))
    nc.all_engine_barrier()


def _build_nc():
    import concourse.bass as bass
    import concourse.tile as tile
    from concourse import mybir

    f32 = mybir.dt.float32
    Alu = mybir.AluOpType
    Act = mybir.ActivationFunctionType

    nc = bass.Bass()
    th = nc.dram_tensor("th", [2, 130], f32, kind="ExternalInput")
    masks = nc.dram_tensor("masks", [128, AW], f32, kind="ExternalInput")
    data = nc.dram_tensor("data", [128, DW], f32, kind="ExternalInput")
    outs = nc.dram_tensor("outs", [128, DW], f32, kind="ExternalOutput")
    outc = nc.dram_tensor("outc", [K, K], f32, kind="ExternalOutput")

    tile.TileContext._drain_and_barrier = _patched_drain_and_barrier
    with tile.TileContext(nc) as tc:
        with (
            tc.tile_pool(name="const", bufs=1) as cp,
            tc.tile_pool(name="work", bufs=1) as wp,
            tc.tile_pool(name="ps", bufs=1, space=bass.MemorySpace.PSUM) as ps,
        ):
            # Loads: butterfly data first on the SP ring (it spins up ~1us
            # earlier than the ACT ring); theta then masks on the ACT ring.
            dt = cp.tile([128, DW], f32, tag="dt")
            nc.sync.dma_start(out=dt[:], in_=data[:])
            tht = cp.tile([2, 130], f32, tag="tht")
            nc.scalar.dma_start(out=tht[:], in_=th[:])
            at = cp.tile([128, AW], f32, tag="at")
            nc.scalar.dma_start(out=at[:], in_=masks[:])

            # Replicate (theta, theta+pi/2) to all 128 partitions: rank-1
            # matmul ones[128] x theta.  Also the PE's th-lane absorber.
            pth = ps.tile([128, 2], f32, tag="pth")
            nc.tensor.matmul(pth[:], tht[:, 2:130], tht[:, 0:2], start=True, stop=True)
            # Absorber for the masks lane: after it the PE reads `at` freely.
            pa = ps.tile([K, K], f32, tag="abs")
            nc.tensor.matmul(pa[:], at[0:K, 0:K], at[0:K, 0:K], start=True, stop=True)

            # (s, c) = sin(theta, theta+pi/2) in one ACT op, staged for DVE.
            acts = cp.tile([128, 2], f32, tag="acts")
            nc.scalar.activation(acts[:], pth[:], Act.Sin)
            sc = cp.tile([128, 2], f32, tag="sc")
            nc.vector.tensor_copy(sc[:], acts[:])

            # DVE copy of the mask head absorbs the masks-DMA wait for DVE.
            mk = cp.tile([K, 192], f32, tag="mk")
            nc.vector.tensor_copy(mk[:], at[0:K, 0:192])

            # n = B^T = s*sinm + idm - c*cosm   (cosm antisymmetric)
            tmp = cp.tile([K, K], f32, tag="tmp")
            nc.vector.scalar_tensor_tensor(tmp[:], mk[:, 64:128], sc[0:K, 0:1], mk[:, 128:192], Alu.mult, Alu.add)
            w = cp.tile([K, K], f32, tag="w")
            nc.vector.tensor_scalar_mul(w[:], mk[:, 0:64], sc[0:K, 1:2])
            n = cp.tile([K, K], f32, tag="n")
            nc.vector.tensor_sub(n[:], tmp[:], w[:])

            # Corner: py = corner^T @ B^T, py2 its partition-pair swap (via
            # the swapped host pack); outc = s*py + (+-c)*py2, all from PSUM.
            # py2 is emitted FIRST so r1's single PE wait (on the later py)
            # also covers py2, keeping the final combine at one sem wait.
            py2 = ps.tile([K, K], f32, tag="y2")
            nc.tensor.matmul(py2[:], at[0:K, 256:320], n[:], start=True, stop=True)
            py = ps.tile([K, K], f32, tag="y")
            nc.tensor.matmul(py[:], at[0:K, 192:256], n[:], start=True, stop=True)

            # Butterfly: outE = s*E + c*O ; outO = s*O - c*E
            osb = wp.tile([128, DW], f32, tag="osb")
            q1 = wp.tile([128, HW], f32, tag="q1")
            nc.vector.tensor_scalar_mul(q1[:], dt[:, 0:HW], sc[:, 0:1])
            nc.vector.scalar_tensor_tensor(osb[:, 0:HW], dt[:, HW:DW], sc[:, 1:2], q1[:], Alu.mult, Alu.add)
            q3 = wp.tile([128, HW], f32, tag="q3")
            nc.vector.tensor_scalar_mul(q3[:], dt[:, 0:HW], sc[:, 1:2])
            nc.vector.scalar_tensor_tensor(osb[:, HW:DW], dt[:, HW:DW], sc[:, 0:1], q3[:], Alu.mult, Alu.subtract)

            # Stores: even plane on the ACT ring, odd plane on the SP ring.
            nc.scalar.dma_start(out=outs[:, 0:HW], in_=osb[:, 0:HW])
            nc.sync.dma_start(out=outs[:, HW:DW], in_=osb[:, HW:DW])

            # Corner combine + store (ACT ring).
            cc = cp.tile([K, 1], f32, tag="cc")
            nc.vector.tensor_scalar_mul(cc[:], at[0:K, 320:321], sc[0:K, 1:2])
            r1 = wp.tile([K, K], f32, tag="r1")
            nc.vector.tensor_scalar_mul(r1[:], py[:], sc[0:K, 0:1])
            oc = wp.tile([K, K], f32, tag="oc")
            nc.vector.scalar_tensor_tensor(oc[:], py2[:], cc[:, 0:1], r1[:], Alu.mult, Alu.add)
            nc.scalar.dma_start(out=outc[:], in_=oc[:])

    return nc


def _get_nc():
    if "nc" not in _CACHE:
        _CACHE["nc"] = _build_nc()
    return _CACHE["nc"]


def _in_maps(input_state, angle, cos_matrix, sin_matrix, id_matrix):
    rho = np.ascontiguousarray(np.asarray(input_state, dtype=np.float32))
    assert rho.shape == (N_FULL, N_FULL)
    theta = np.float32(np.asarray(angle))

    corner = lambda m: np.asarray(m, dtype=np.float32)[0:K, 0:K]
    am = np.zeros((128, AW), dtype=np.float32)
    am[0:K, 0:64] = corner(cos_matrix)
    am[0:K, 64:128] = corner(sin_matrix)
    am[0:K, 128:192] = corner(id_matrix)
    am[0:K:2, 320] = 1.0
    am[1:K:2, 320] = -1.0
    tht = np.zeros((2, 130), dtype=np.float32)
    tht[0, 0] = theta
    tht[0, 1] = theta + np.float32(np.pi / 2)
    tht[0, 2:130] = 1.0

    maps = []
    for c in range(N_CORES):
        a = am if c else am.copy()
        if c == 0:
            a[0:K, 192:256] = rho[0:K, 0:K]
            sw = rho[0:K, 0:K].reshape(K, K // 2, 2)[:, :, ::-1].reshape(K, K)
            a[0:K, 256:320] = sw
        pos = np.zeros((NG * 128, K), dtype=np.float32)
        pos[0:RW] = rho[0:K, K + c * RW : K + (c + 1) * RW].T
        pos[RW:NPOS] = rho[K + c * RW : K + (c + 1) * RW, 0:K]
        # [1024, 64] -> per-group packing [128, 8*32] for even/odd planes
        ev = pos[:, 0::2].reshape(NG, 128, K // 2).transpose(1, 0, 2).reshape(128, HW)
        od = pos[:, 1::2].reshape(NG, 128, K // 2).transpose(1, 0, 2).reshape(128, HW)
        d = np.empty((128, DW), dtype=np.float32)
        d[:, 0:HW] = ev
        d[:, HW:DW] = od
        maps.append({"th": tht, "masks": a, "data": d})
    return maps


def _assemble(input_state, results):
    full = np.array(np.asarray(input_state, dtype=np.float32), copy=True)
    vals = np.empty((NG * 128, K), dtype=np.float32)
    for c in range(N_CORES):
        o = results[c]["outs"]
        vals[:, 0::2] = o[:, 0:HW].reshape(128, NG, K // 2).transpose(1, 0, 2).reshape(NG * 128, K // 2)
        vals[:, 1::2] = o[:, HW:DW].reshape(128, NG, K // 2).transpose(1, 0, 2).reshape(NG * 128, K // 2)
        full[0:K, K + c * RW : K + (c + 1) * RW] = vals[0:RW].T
        full[K + c * RW : K + (c + 1) * RW, 0:K] = vals[RW:NPOS]
    full[0:K, 0:K] = results[0]["outc"].T
    return full


def run(input_state, angle, cos_matrix, sin_matrix, id_matrix, **spmd_kwargs):
    from concourse.bass_utils import run_bass_kernel_spmd

    nc = _get_nc()
    maps = _in_maps(input_state, angle, cos_matrix, sin_matrix, id_matrix)
    res = run_bass_kernel_spmd(nc, maps, list(range(N_CORES)), **spmd_kwargs)
    return _assemble(input_state, res.results).astype(np.float32, copy=False), res


def kernel(input_state, angle, cos_matrix, sin_matrix, id_matrix):
    full, _ = run(input_state, angle, cos_matrix, sin_matrix, id_matrix)
    return full


# revision 17
# speedup vs baseline: 1.1122x; 1.0592x over previous
"""Trainium2 kernel for nn_BS_Registers_density: out = U @ rho @ U.T.

U = cos(a)*cos_mask + sin(a)*sin_mask + id_mask is the identity outside its
top-left 64x64 corner (32 disjoint 2x2 Givens blocks), so the product only
modifies the first 64 rows and first 64 columns of rho:

  out[0:64,  64:] = B @ rho[0:64, 64:]          (row strip)
  out[64:,  0:64] = rho[64:, 0:64] @ B^T        (col strip)
  out[0:64, 0:64] = B @ rho[0:64, 0:64] @ B^T   (corner)
  out[64:,  64:]  = rho[64:, 64:]               (identity pass-through)

with B = U[0:64, 0:64].  Sharding (per the hint: "the sparse rotation
structure allows replicating only the k affected rows"): each core receives
ONLY the affected data — a 504-column slice of the k=64 affected rows plus a
504-row slice of the affected column strip.  The device computes every
changed output element; the identity pass-through block never transits the
device — the host unshard step pastes the device-computed strips into a copy
of rho (data movement only, no host arithmetic; theta -> sin/cos and all
products happen on device).

Both strip updates are the same 2x2 butterfly on adjacent lanes l=2k,2k+1:

  out[2k] = sin*in[2k] + cos*in[2k+1],  out[2k+1] = -cos*in[2k] + sin*in[2k+1]

row strip: positions = columns, lanes = the 64 affected rows (host packs
transposed); col strip: positions = rows, lanes = the 64 affected columns
(natural layout).  The host splits lanes into even/odd planes so the whole
update is 4 contiguous elementwise ops on [128, 256] tiles — the ACT engine
does the scalings (activation Copy with a per-partition scale AP), the DVE
the fused combine — exact fp32 2-term arithmetic, no PE serialization.  The
64x64 corner runs concurrently on the PE: two matmuls via the lhsT-transpose
trick (the second on a host-packed column-pair-swapped corner, turning the
partition-pair mix into an elementwise DVE combine of the two PSUM tiles);
cores 1-7 compute a zero corner the host ignores.

Latency details (measured in prior sessions):
  - theta rides a tiny [2, 130] tensor (theta, theta+pi/2, a row of ones) at
    the head of the SP ring; ACT computes sin on those 2 partitions and one
    rank-1 PE matmul (ones x sin) replicates to 128 partitions — a [128, 2]
    theta tensor would be 128 8-byte DMA descriptors (~3us);
  - every instruction encodes at most ONE NEW semaphore wait; tiny absorber
    ops let each engine observe each DMA lane / producer engine once;
  - the kernel-tail Drain cannot carry one wait per live semaphore, so the
    patched tail below spreads them across SP no-ops; the stock tail's
    semaphore clears + second barrier are skipped — the NEFF epilogue
    re-zeroes all 256 hardware semaphores regardless;
  - only 2 HWDGE rings exist (SP + ACT); the SP ring spins up ~1us earlier,
    so the big loads ride it; 6 DMAs total (8 lanes available).
"""

import numpy as np

N_CORES = 8
N_FULL = 4096
K = 64  # size of the affected corner block
RW = (N_FULL - K) // N_CORES  # 504: strip positions per core per strip
NG = 8  # position groups of 128
HW = NG * (K // 2)  # 256: even (or odd) lane-plane width
DW = 2 * HW  # 512

# masks tensor layout (f32, [64, AW]):
#   cols    0:64   real cos mask          col   192      parity (+1/-1)
#   cols   64:128  real sin mask          cols  193:257  corner_in (core 0)
#   cols  128:192  real id mask           cols  257:321  corner col-pair-swapped
AW = 321

# th tensor (f32, [2, 130]): row 0 = (theta, theta+pi/2, ones[128]), row 1 = 0
# data tensor (f32, [128, 512]): cols 0:256 even lanes, 256:512 odd lanes;
# position p = g*128 + partition: p < 504 -> row strip (transposed col
# slice), 504 <= p < 1008 -> col strip (natural row slice), rest pad.

_CACHE = {}


def _patched_drain_and_barrier(self, tick_clock, wait_clock):
    """Kernel-tail replacement for TileContext._drain_and_barrier.

    The stock tail attaches every outstanding semaphore wait to one Drain
    instruction, but the TRN2 instruction encoding holds a single semaphore
    wait, so walrus rejects it ("Too many sync wait commands").  Spread the
    waits across one SP no-op per semaphore instead, then drain + barrier.
    The stock clear_and_free_semaphores + second barrier are skipped: the
    NEFF epilogue zeroes all 256 hardware semaphores after the program, and
    the preamble of the next execution resets the kernel range again.
    """
    import re

    import bass_rust
    from concourse.vector_clock import ScopedClock

    nc = self.nc
    vals = [int(x) for x in re.findall(r"\d+", repr(tick_clock.global_clock))]
    for proc, val in enumerate(vals):
        if val <= 0:
            continue
        nop = nc.sync.nop()
        mask = bass_rust.VectorClock()
        mask.require_at_least(proc, val)
        wait_clock.add_sem_waits(nop.ins, ScopedClock({None: mask}))

    nc.sync.drain()
    nc.all_engine_barrier()
    popped = nc._tile_sem_poison_stack.pop()
    assert popped is self._sem_poison
    self.sems.allocated()


def _build_nc():
    import concourse.bass as bass
    import concourse.tile as tile
    from concourse import mybir

    f32 = mybir.dt.float32
    Alu = mybir.AluOpType
    Act = mybir.ActivationFunctionType

    nc = bass.Bass()
    th = nc.dram_tensor("th", [2, 130], f32, kind="ExternalInput")
    masks = nc.dram_tensor("masks", [K, AW], f32, kind="ExternalInput")
    data = nc.dram_tensor("data", [128, DW], f32, kind="ExternalInput")
    outs = nc.dram_tensor("outs", [128, DW], f32, kind="ExternalOutput")
    outc = nc.dram_tensor("outc", [K, K], f32, kind="ExternalOutput")

    tile.TileContext._drain_and_barrier = _patched_drain_and_barrier
    with tile.TileContext(nc) as tc:
        with (
            tc.tile_pool(name="const", bufs=1) as cp,
            tc.tile_pool(name="work", bufs=1) as wp,
            tc.tile_pool(name="ps", bufs=1, space=bass.MemorySpace.PSUM) as ps,
        ):
            # Loads: tiny theta then butterfly data on the SP ring (spins up
            # ~1us earlier); masks for the corner path on the ACT ring.
            tht = cp.tile([2, 130], f32, tag="tht")
            nc.sync.dma_start(out=tht[:], in_=th[:])
            dt = cp.tile([128, DW], f32, tag="dt")
            nc.sync.dma_start(out=dt[:], in_=data[:])
            at = cp.tile([K, AW], f32, tag="at")
            nc.scalar.dma_start(out=at[:], in_=masks[:])

            # (s, c) on 2 partitions straight off the theta tile, then one
            # rank-1 matmul (ones x sc2) replicates to 128 partitions.  The
            # tiny pa2 matmul absorbs the theta lane for the PE.
            acts2 = cp.tile([2, 2], f32, tag="acts2")
            nc.scalar.activation(acts2[:], tht[:, 0:2], Act.Sin)
            pa2 = ps.tile([2, 2], f32, tag="abs2")
            nc.tensor.matmul(pa2[:], tht[:, 0:2], tht[:, 0:2], start=True, stop=True)
            pth = ps.tile([128, 2], f32, tag="pth")
            nc.tensor.matmul(pth[:], tht[:, 2:130], acts2[:], start=True, stop=True)

            # DVE stages the coefficients to SBUF (PE absorber for DVE).
            sc = cp.tile([128, 2], f32, tag="sc")
            nc.vector.tensor_copy(sc[:], pth[:])

            # ACT: absorb the DVE tick, then the two big scalings
            # q1 = s*E, q3 = c*E (activation Copy, per-partition scale AP —
            # the scale must live in SBUF, hence sc not pth).
            absp = cp.tile([2, 2], f32, tag="absp")
            nc.scalar.activation(absp[:], sc[0:2, 0:2], Act.Copy)
            q1 = wp.tile([128, HW], f32, tag="q1")
            nc.scalar.activation(q1[:], dt[:, 0:HW], Act.Copy, scale=sc[:, 0:1])
            q3 = wp.tile([128, HW], f32, tag="q3")
            nc.scalar.activation(q3[:], dt[:, 0:HW], Act.Copy, scale=sc[:, 1:2])

            mk = cp.tile([K, 193], f32, tag="mk")
            nc.vector.tensor_copy(mk[:], at[:, 0:193])
            # PE absorber for the masks lane (so py2/py carry only the DVE wait)
            pa = ps.tile([K, K], f32, tag="abs")
            nc.tensor.matmul(pa[:], at[:, 0:K], at[:, 0:K], start=True, stop=True)
            tmp = cp.tile([K, K], f32, tag="tmp")
            nc.vector.scalar_tensor_tensor(tmp[:], mk[:, 64:128], sc[0:K, 0:1], mk[:, 128:192], Alu.mult, Alu.add)
            w = cp.tile([K, K], f32, tag="w")
            nc.vector.tensor_scalar_mul(w[:], mk[:, 0:64], sc[0:K, 1:2])
            n = cp.tile([K, K], f32, tag="n")
            nc.vector.tensor_sub(n[:], tmp[:], w[:])
            pv = cp.tile([K, 1], f32, tag="pv")
            nc.vector.tensor_copy(pv[:], dt[0:K, 0:1])

            # Corner on the PE: py2 first so r1's single PE wait (on the
            # later py) also covers it.
            py2 = ps.tile([K, K], f32, tag="y2")
            nc.tensor.matmul(py2[:], at[:, 257:321], n[:], start=True, stop=True)
            py = ps.tile([K, K], f32, tag="y")
            nc.tensor.matmul(py[:], at[:, 193:257], n[:], start=True, stop=True)

            # Butterfly combines on DVE: outE = c*O + q1, outO = s*O - q3.
            osbe = wp.tile([128, HW], f32, tag="osbe")
            nc.vector.scalar_tensor_tensor(osbe[:], dt[:, HW:DW], sc[:, 1:2], q1[:], Alu.mult, Alu.add)
            osbo = wp.tile([128, HW], f32, tag="osbo")
            nc.vector.scalar_tensor_tensor(osbo[:], dt[:, HW:DW], sc[:, 0:1], q3[:], Alu.mult, Alu.subtract)

            # Stores: even plane on the ACT ring, odd plane on the SP ring.
            nc.scalar.dma_start(out=outs[:, 0:HW], in_=osbe[:])
            nc.sync.dma_start(out=outs[:, HW:DW], in_=osbo[:])

            # Corner combine + store (ACT ring): outc = s*py + (+-c)*py2.
            ccv = cp.tile([K, 1], f32, tag="ccv")
            nc.vector.tensor_scalar_mul(ccv[:], mk[:, 192:193], sc[0:K, 1:2])
            r1 = wp.tile([K, K], f32, tag="r1")
            nc.vector.tensor_scalar_mul(r1[:], py[:], sc[0:K, 0:1])
            oc = wp.tile([K, K], f32, tag="oc")
            nc.vector.scalar_tensor_tensor(oc[:], py2[:], ccv[:, 0:1], r1[:], Alu.mult, Alu.add)
            nc.scalar.dma_start(out=outc[:], in_=oc[:])

    return nc


def _get_nc():
    if "nc" not in _CACHE:
        _CACHE["nc"] = _build_nc()
    return _CACHE["nc"]


def _in_maps(input_state, angle, cos_matrix, sin_matrix, id_matrix):
    rho = np.ascontiguousarray(np.asarray(input_state, dtype=np.float32))
    assert rho.shape == (N_FULL, N_FULL)
    theta = np.float32(np.asarray(angle))

    corner = lambda m: np.asarray(m, dtype=np.float32)[0:K, 0:K]
    am = np.zeros((K, AW), dtype=np.float32)
    am[:, 0:64] = corner(cos_matrix)
    am[:, 64:128] = corner(sin_matrix)
    am[:, 128:192] = corner(id_matrix)
    am[0:K:2, 192] = 1.0
    am[1:K:2, 192] = -1.0
    tht = np.zeros((2, 130), dtype=np.float32)
    tht[0, 0] = theta
    tht[0, 1] = theta + np.float32(np.pi / 2)
    tht[0, 2:130] = 1.0

    maps = []
    for c in range(N_CORES):
        a = am if c else am.copy()
        if c == 0:
            a[:, 193:257] = rho[0:K, 0:K]
            a[:, 257:321] = rho[0:K, 0:K].reshape(K, K // 2, 2)[:, :, ::-1].reshape(K, K)
        pos = np.zeros((NG * 128, K), dtype=np.float32)
        pos[0:RW] = rho[0:K, K + c * RW : K + (c + 1) * RW].T
        pos[RW : 2 * RW] = rho[K + c * RW : K + (c + 1) * RW, 0:K]
        # [1024, 64] -> per-group packing [128, NG*32] for even/odd planes
        ev = pos[:, 0::2].reshape(NG, 128, K // 2).transpose(1, 0, 2).reshape(128, HW)
        od = pos[:, 1::2].reshape(NG, 128, K // 2).transpose(1, 0, 2).reshape(128, HW)
        d = np.empty((128, DW), dtype=np.float32)
        d[:, 0:HW] = ev
        d[:, HW:DW] = od
        maps.append({"th": tht, "masks": a, "data": d})
    return maps


def _assemble(input_state, results):
    full = np.array(np.asarray(input_state, dtype=np.float32), copy=True)
    vals = np.empty((NG * 128, K), dtype=np.float32)
    for c in range(N_CORES):
        o = results[c]["outs"]
        vals[:, 0::2] = o[:, 0:HW].reshape(128, NG, K // 2).transpose(1, 0, 2).reshape(NG * 128, K // 2)
        vals[:, 1::2] = o[:, HW:DW].reshape(128, NG, K // 2).transpose(1, 0, 2).reshape(NG * 128, K // 2)
        full[0:K, K + c * RW : K + (c + 1) * RW] = vals[0:RW].T
        full[K + c * RW : K + (c + 1) * RW, 0:K] = vals[RW : 2 * RW]
    full[0:K, 0:K] = results[0]["outc"].T
    return full


def run(input_state, angle, cos_matrix, sin_matrix, id_matrix, **spmd_kwargs):
    from concourse.bass_utils import run_bass_kernel_spmd

    nc = _get_nc()
    maps = _in_maps(input_state, angle, cos_matrix, sin_matrix, id_matrix)
    res = run_bass_kernel_spmd(nc, maps, list(range(N_CORES)), **spmd_kwargs)
    return _assemble(input_state, res.results).astype(np.float32, copy=False), res


def kernel(input_state, angle, cos_matrix, sin_matrix, id_matrix):
    full, _ = run(input_state, angle, cos_matrix, sin_matrix, id_matrix)
    return full


# revision 19
# speedup vs baseline: 1.1743x; 1.0558x over previous
"""Trainium2 kernel for nn_BS_Registers_density: out = U @ rho @ U.T.

U = cos(a)*cos_mask + sin(a)*sin_mask + id_mask is the identity outside its
top-left 64x64 corner (32 disjoint 2x2 Givens blocks), so the product only
modifies the first 64 rows and first 64 columns of rho:

  out[0:64,  64:] = B @ rho[0:64, 64:]          (row strip)
  out[64:,  0:64] = rho[64:, 0:64] @ B^T        (col strip)
  out[0:64, 0:64] = B @ rho[0:64, 0:64] @ B^T   (corner)
  out[64:,  64:]  = rho[64:, 64:]               (identity pass-through)

with B = U[0:64, 0:64].  Sharding (per the hint: "the sparse rotation
structure allows replicating only the k affected rows"): each core receives
ONLY the affected data — a 504-column slice of the k=64 affected rows plus a
504-row slice of the affected column strip.  The device computes every
changed output element; the identity pass-through block never transits the
device — the host unshard step pastes the device-computed strips into a copy
of rho (data movement only, no host arithmetic; theta -> sin/cos and all
products happen on device).

Both strip updates are the same 2x2 butterfly on adjacent lanes l=2k,2k+1:

  out[2k] = sin*in[2k] + cos*in[2k+1],  out[2k+1] = -cos*in[2k] + sin*in[2k+1]

row strip: positions = columns, lanes = the 64 affected rows (host packs
transposed); col strip: positions = rows, lanes = the 64 affected columns
(natural layout).  The host splits lanes into even/odd planes so the whole
update is 4 contiguous elementwise ops on [128, 256] tiles — the ACT engine
does the scalings (activation Copy with a per-partition scale AP), the DVE
the fused combine — exact fp32 2-term arithmetic, no PE serialization.  The
64x64 corner runs concurrently on the PE: two matmuls via the lhsT-transpose
trick (the second on a host-packed column-pair-swapped corner, turning the
partition-pair mix into an elementwise DVE combine of the two PSUM tiles);
cores 1-7 compute a zero corner the host ignores.

Latency details (measured in prior sessions):
  - theta rides a tiny [2, 130] tensor (theta, theta+pi/2, a row of ones) at
    the head of the SP ring; ACT computes sin on those 2 partitions and one
    rank-1 PE matmul (ones x sin) replicates to 128 partitions — a [128, 2]
    theta tensor would be 128 8-byte DMA descriptors (~3us);
  - every instruction encodes at most ONE NEW semaphore wait; tiny absorber
    ops let each engine observe each DMA lane / producer engine once;
  - the kernel-tail Drain cannot carry one wait per live semaphore, so the
    patched tail below spreads them across SP no-ops; the stock tail's
    semaphore clears + second barrier are skipped — the NEFF epilogue
    re-zeroes all 256 hardware semaphores regardless;
  - only 2 HWDGE rings exist (SP + ACT); the SP ring spins up ~1us earlier,
    so the big loads ride it; 6 DMAs total (8 lanes available).
"""

import numpy as np

N_CORES = 8
N_FULL = 4096
K = 64  # size of the affected corner block
RW = (N_FULL - K) // N_CORES  # 504: strip positions per core per strip
NG = 8  # position groups of 128
HW = NG * (K // 2)  # 256: even (or odd) lane-plane width
DW = 2 * HW  # 512

# masks tensor layout (f32, [64, AW]):
#   cols    0:64   real cos mask          col   192      parity (+1/-1)
#   cols   64:128  real sin mask          cols  193:257  corner_in (core 0)
#   cols  128:192  real id mask           cols  257:321  corner col-pair-swapped
AW = 321

# th tensor (f32, [2, 130]): row 0 = (theta, theta+pi/2, ones[128]), row 1 = 0
# data tensor (f32, [128, 512]): cols 0:256 even lanes, 256:512 odd lanes;
# position p = g*128 + partition: p < 504 -> row strip (transposed col
# slice), 504 <= p < 1008 -> col strip (natural row slice), rest pad.

_CACHE = {}


def _patched_drain_and_barrier(self, tick_clock, wait_clock):
    """Kernel-tail replacement for TileContext._drain_and_barrier.

    The stock tail attaches every outstanding semaphore wait to one Drain
    instruction, but the TRN2 instruction encoding holds a single semaphore
    wait, so walrus rejects it ("Too many sync wait commands").  Spread the
    waits across one SP no-op per semaphore instead, then drain + barrier.
    The stock clear_and_free_semaphores + second barrier are skipped: the
    NEFF epilogue zeroes all 256 hardware semaphores after the program, and
    the preamble of the next execution resets the kernel range again.
    """
    import re

    import bass_rust
    from concourse.vector_clock import ScopedClock

    nc = self.nc
    vals = [int(x) for x in re.findall(r"\d+", repr(tick_clock.global_clock))]
    for proc, val in enumerate(vals):
        if val <= 0:
            continue
        nop = nc.sync.nop()
        mask = bass_rust.VectorClock()
        mask.require_at_least(proc, val)
        wait_clock.add_sem_waits(nop.ins, ScopedClock({None: mask}))

    nc.sync.drain()
    nc.all_engine_barrier()
    popped = nc._tile_sem_poison_stack.pop()
    assert popped is self._sem_poison
    self.sems.allocated()


def _build_nc():
    import concourse.bass as bass
    import concourse.tile as tile
    from concourse import mybir

    f32 = mybir.dt.float32
    Alu = mybir.AluOpType
    Act = mybir.ActivationFunctionType

    nc = bass.Bass()
    th = nc.dram_tensor("th", [2, 130], f32, kind="ExternalInput")
    masks = nc.dram_tensor("masks", [K, AW], f32, kind="ExternalInput")
    data = nc.dram_tensor("data", [128, DW], f32, kind="ExternalInput")
    outs = nc.dram_tensor("outs", [128, DW], f32, kind="ExternalOutput")
    outc = nc.dram_tensor("outc", [K, K], f32, kind="ExternalOutput")

    tile.TileContext._drain_and_barrier = _patched_drain_and_barrier
    with tile.TileContext(nc) as tc:
        with (
            tc.tile_pool(name="const", bufs=1) as cp,
            tc.tile_pool(name="work", bufs=1) as wp,
            tc.tile_pool(name="ps", bufs=1, space=bass.MemorySpace.PSUM) as ps,
        ):
            # Loads: tiny theta then butterfly data on the SP ring (spins up
            # ~1us earlier); masks for the corner path on the ACT ring.
            tht = cp.tile([2, 130], f32, tag="tht")
            nc.sync.dma_start(out=tht[:], in_=th[:])
            dt = cp.tile([128, DW], f32, tag="dt")
            nc.sync.dma_start(out=dt[:], in_=data[:])
            at = cp.tile([K, AW], f32, tag="at")
            nc.scalar.dma_start(out=at[:], in_=masks[:])

            # Replicate RAW theta to 128 partitions the moment it lands
            # (rank-1 PE matmul, no ACT dependency; pa2 absorbs the theta
            # lane), then one 128-partition Sin straight off PSUM gives
            # (s, c) = (sin, cos) with a single cross-engine hop.
            pa2 = ps.tile([2, 2], f32, tag="abs2")
            nc.tensor.matmul(pa2[:], tht[:, 0:2], tht[:, 0:2], start=True, stop=True)
            ptht = ps.tile([128, 2], f32, tag="ptht")
            nc.tensor.matmul(ptht[:], tht[:, 2:130], tht[:, 0:2], start=True, stop=True)
            acts = cp.tile([128, 2], f32, tag="acts")
            nc.scalar.activation(acts[:], ptht[:], Act.Sin)
            absq = cp.tile([2, 2], f32, tag="absq")
            nc.scalar.activation(absq[:], acts[0:2, 0:2], Act.Copy)
            q1 = wp.tile([128, HW], f32, tag="q1")
            nc.scalar.activation(q1[:], dt[:, 0:HW], Act.Copy, scale=acts[:, 0:1])

            # DVE staging copy absorbs the ACT tick for DVE.
            scd = cp.tile([128, 2], f32, tag="scd")
            nc.vector.tensor_copy(scd[:], acts[:])

            mk = cp.tile([K, 193], f32, tag="mk")
            nc.vector.tensor_copy(mk[:], at[:, 0:193])
            # PE absorber for the masks lane (so py2/py carry only the DVE wait)
            pa = ps.tile([K, K], f32, tag="abs")
            nc.tensor.matmul(pa[:], at[:, 0:K], at[:, 0:K], start=True, stop=True)
            tmp = cp.tile([K, K], f32, tag="tmp")
            nc.vector.scalar_tensor_tensor(tmp[:], mk[:, 64:128], scd[0:K, 0:1], mk[:, 128:192], Alu.mult, Alu.add)
            w = cp.tile([K, K], f32, tag="w")
            nc.vector.tensor_scalar_mul(w[:], mk[:, 0:64], scd[0:K, 1:2])
            n = cp.tile([K, K], f32, tag="n")
            nc.vector.tensor_sub(n[:], tmp[:], w[:])

            # Corner on the PE: py2 first so r1's single PE wait (on the
            # later py) also covers it.
            py2 = ps.tile([K, K], f32, tag="y2")
            nc.tensor.matmul(py2[:], at[:, 257:321], n[:], start=True, stop=True)
            py = ps.tile([K, K], f32, tag="y")
            nc.tensor.matmul(py[:], at[:, 193:257], n[:], start=True, stop=True)

            # Odd plane entirely on DVE: q3 = c*E, outO = s*O - q3; the even
            # combine outE = c*O + q1 follows once ACT's q1 lands.
            q3 = wp.tile([128, HW], f32, tag="q3")
            nc.vector.tensor_scalar_mul(q3[:], dt[:, 0:HW], scd[:, 1:2])
            osbo = wp.tile([128, HW], f32, tag="osbo")
            nc.vector.scalar_tensor_tensor(osbo[:], dt[:, HW:DW], scd[:, 0:1], q3[:], Alu.mult, Alu.subtract)
            osbe = wp.tile([128, HW], f32, tag="osbe")
            nc.vector.scalar_tensor_tensor(osbe[:], dt[:, HW:DW], scd[:, 1:2], q1[:], Alu.mult, Alu.add)

            # Stores: odd plane on the SP ring, even plane on the ACT ring.
            nc.sync.dma_start(out=outs[:, HW:DW], in_=osbo[:])
            nc.scalar.dma_start(out=outs[:, 0:HW], in_=osbe[:])

            # Corner combine + store (ACT ring): outc = s*py + (+-c)*py2.
            ccv = cp.tile([K, 1], f32, tag="ccv")
            nc.vector.tensor_scalar_mul(ccv[:], mk[:, 192:193], scd[0:K, 1:2])
            r1 = wp.tile([K, K], f32, tag="r1")
            nc.vector.tensor_scalar_mul(r1[:], py[:], scd[0:K, 0:1])
            oc = wp.tile([K, K], f32, tag="oc")
            nc.vector.scalar_tensor_tensor(oc[:], py2[:], ccv[:, 0:1], r1[:], Alu.mult, Alu.add)
            nc.scalar.dma_start(out=outc[:], in_=oc[:])

    return nc


def _get_nc():
    if "nc" not in _CACHE:
        _CACHE["nc"] = _build_nc()
    return _CACHE["nc"]


def _in_maps(input_state, angle, cos_matrix, sin_matrix, id_matrix):
    rho = np.ascontiguousarray(np.asarray(input_state, dtype=np.float32))
    assert rho.shape == (N_FULL, N_FULL)
    theta = np.float32(np.asarray(angle))

    corner = lambda m: np.asarray(m, dtype=np.float32)[0:K, 0:K]
    am = np.zeros((K, AW), dtype=np.float32)
    am[:, 0:64] = corner(cos_matrix)
    am[:, 64:128] = corner(sin_matrix)
    am[:, 128:192] = corner(id_matrix)
    am[0:K:2, 192] = 1.0
    am[1:K:2, 192] = -1.0
    tht = np.zeros((2, 130), dtype=np.float32)
    tht[0, 0] = theta
    tht[0, 1] = theta + np.float32(np.pi / 2)
    tht[0, 2:130] = 1.0

    maps = []
    for c in range(N_CORES):
        a = am if c else am.copy()
        if c == 0:
            a[:, 193:257] = rho[0:K, 0:K]
            a[:, 257:321] = rho[0:K, 0:K].reshape(K, K // 2, 2)[:, :, ::-1].reshape(K, K)
        pos = np.zeros((NG * 128, K), dtype=np.float32)
        pos[0:RW] = rho[0:K, K + c * RW : K + (c + 1) * RW].T
        pos[RW : 2 * RW] = rho[K + c * RW : K + (c + 1) * RW, 0:K]
        # [1024, 64] -> per-group packing [128, NG*32] for even/odd planes
        ev = pos[:, 0::2].reshape(NG, 128, K // 2).transpose(1, 0, 2).reshape(128, HW)
        od = pos[:, 1::2].reshape(NG, 128, K // 2).transpose(1, 0, 2).reshape(128, HW)
        d = np.empty((128, DW), dtype=np.float32)
        d[:, 0:HW] = ev
        d[:, HW:DW] = od
        maps.append({"th": tht, "masks": a, "data": d})
    return maps


def _assemble(input_state, results):
    full = np.array(np.asarray(input_state, dtype=np.float32), copy=True)
    vals = np.empty((NG * 128, K), dtype=np.float32)
    for c in range(N_CORES):
        o = results[c]["outs"]
        vals[:, 0::2] = o[:, 0:HW].reshape(128, NG, K // 2).transpose(1, 0, 2).reshape(NG * 128, K // 2)
        vals[:, 1::2] = o[:, HW:DW].reshape(128, NG, K // 2).transpose(1, 0, 2).reshape(NG * 128, K // 2)
        full[0:K, K + c * RW : K + (c + 1) * RW] = vals[0:RW].T
        full[K + c * RW : K + (c + 1) * RW, 0:K] = vals[RW : 2 * RW]
    full[0:K, 0:K] = results[0]["outc"].T
    return full


def run(input_state, angle, cos_matrix, sin_matrix, id_matrix, **spmd_kwargs):
    from concourse.bass_utils import run_bass_kernel_spmd

    nc = _get_nc()
    maps = _in_maps(input_state, angle, cos_matrix, sin_matrix, id_matrix)
    res = run_bass_kernel_spmd(nc, maps, list(range(N_CORES)), **spmd_kwargs)
    return _assemble(input_state, res.results).astype(np.float32, copy=False), res


def kernel(input_state, angle, cos_matrix, sin_matrix, id_matrix):
    full, _ = run(input_state, angle, cos_matrix, sin_matrix, id_matrix)
    return full


# revision 20
# speedup vs baseline: 1.1837x; 1.0081x over previous
"""Trainium2 kernel for nn_BS_Registers_density: out = U @ rho @ U.T.

U = cos(a)*cos_mask + sin(a)*sin_mask + id_mask is the identity outside its
top-left 64x64 corner (32 disjoint 2x2 Givens blocks), so the product only
modifies the first 64 rows and first 64 columns of rho:

  out[0:64,  64:] = B @ rho[0:64, 64:]          (row strip)
  out[64:,  0:64] = rho[64:, 0:64] @ B^T        (col strip)
  out[0:64, 0:64] = B @ rho[0:64, 0:64] @ B^T   (corner)
  out[64:,  64:]  = rho[64:, 64:]               (identity pass-through)

with B = U[0:64, 0:64].  Sharding (per the hint: "the sparse rotation
structure allows replicating only the k affected rows"): each core receives
ONLY the affected data — a 504-column slice of the k=64 affected rows plus a
504-row slice of the affected column strip.  The device computes every
changed output element; the identity pass-through block never transits the
device — the host unshard step pastes the device-computed strips into a copy
of rho (data movement only, no host arithmetic; theta -> sin/cos and all
products happen on device).

Both strip updates are the same 2x2 butterfly on adjacent lanes l=2k,2k+1:

  out[2k] = sin*in[2k] + cos*in[2k+1],  out[2k+1] = -cos*in[2k] + sin*in[2k+1]

row strip: positions = columns, lanes = the 64 affected rows (host packs
transposed); col strip: positions = rows, lanes = the 64 affected columns
(natural layout).  The host splits lanes into even/odd planes so the whole
update is 4 contiguous elementwise ops on [128, 256] tiles — the ACT engine
does the scalings (activation Copy with a per-partition scale AP), the DVE
the fused combine — exact fp32 2-term arithmetic, no PE serialization.  The
64x64 corner runs concurrently on the PE: two matmuls via the lhsT-transpose
trick (the second on a host-packed column-pair-swapped corner, turning the
partition-pair mix into an elementwise DVE combine of the two PSUM tiles);
cores 1-7 compute a zero corner the host ignores.

Latency details (measured in prior sessions):
  - theta rides a tiny [2, 130] tensor (theta, theta+pi/2, a row of ones) at
    the head of the SP ring; ACT computes sin on those 2 partitions and one
    rank-1 PE matmul (ones x sin) replicates to 128 partitions — a [128, 2]
    theta tensor would be 128 8-byte DMA descriptors (~3us);
  - every instruction encodes at most ONE NEW semaphore wait; tiny absorber
    ops let each engine observe each DMA lane / producer engine once;
  - the kernel-tail Drain cannot carry one wait per live semaphore, so the
    patched tail below spreads them across SP no-ops; the stock tail's
    semaphore clears + second barrier are skipped — the NEFF epilogue
    re-zeroes all 256 hardware semaphores regardless;
  - only 2 HWDGE rings exist (SP + ACT); the SP ring spins up ~1us earlier,
    so the big loads ride it; 6 DMAs total (8 lanes available).
"""

import numpy as np

N_CORES = 8
N_FULL = 4096
K = 64  # size of the affected corner block
RW = (N_FULL - K) // N_CORES  # 504: strip positions per core per strip
NG = 8  # position groups of 128
HW = NG * (K // 2)  # 256: even (or odd) lane-plane width
DW = 2 * HW  # 512

# masks tensor layout (f32, [64, AW]):
#   cols    0:64   real cos mask          col   192      parity (+1/-1)
#   cols   64:128  real sin mask          cols  193:257  corner_in (core 0)
#   cols  128:192  real id mask           cols  257:321  corner col-pair-swapped
AW = 321

# th tensor (f32, [2, 130]): row 0 = (theta, theta+pi/2, ones[128]), row 1 = 0
# data tensor (f32, [128, 512]): cols 0:256 even lanes, 256:512 odd lanes;
# position p = g*128 + partition: p < 504 -> row strip (transposed col
# slice), 504 <= p < 1008 -> col strip (natural row slice), rest pad.

_CACHE = {}


def _patched_drain_and_barrier(self, tick_clock, wait_clock):
    """Kernel-tail replacement for TileContext._drain_and_barrier.

    The stock tail attaches every outstanding semaphore wait to one Drain
    instruction, but the TRN2 instruction encoding holds a single semaphore
    wait, so walrus rejects it ("Too many sync wait commands").  Spread the
    waits across one SP no-op per semaphore instead, then drain + barrier.
    The stock clear_and_free_semaphores + second barrier are skipped: the
    NEFF epilogue zeroes all 256 hardware semaphores after the program, and
    the preamble of the next execution resets the kernel range again.
    """
    import re

    import bass_rust
    from concourse.vector_clock import ScopedClock

    nc = self.nc
    vals = [int(x) for x in re.findall(r"\d+", repr(tick_clock.global_clock))]
    for proc, val in enumerate(vals):
        if val <= 0:
            continue
        nop = nc.sync.nop()
        mask = bass_rust.VectorClock()
        mask.require_at_least(proc, val)
        wait_clock.add_sem_waits(nop.ins, ScopedClock({None: mask}))

    nc.sync.drain()
    nc.all_engine_barrier()
    popped = nc._tile_sem_poison_stack.pop()
    assert popped is self._sem_poison
    self.sems.allocated()


def _build_nc():
    import concourse.bass as bass
    import concourse.tile as tile
    from concourse import mybir

    f32 = mybir.dt.float32
    Alu = mybir.AluOpType
    Act = mybir.ActivationFunctionType

    nc = bass.Bass()
    th = nc.dram_tensor("th", [2, 130], f32, kind="ExternalInput")
    masks = nc.dram_tensor("masks", [K, AW], f32, kind="ExternalInput")
    data = nc.dram_tensor("data", [128, DW], f32, kind="ExternalInput")
    outs = nc.dram_tensor("outs", [128, DW], f32, kind="ExternalOutput")
    outc = nc.dram_tensor("outc", [K, K], f32, kind="ExternalOutput")

    tile.TileContext._drain_and_barrier = _patched_drain_and_barrier
    with tile.TileContext(nc) as tc:
        with (
            tc.tile_pool(name="const", bufs=1) as cp,
            tc.tile_pool(name="work", bufs=1) as wp,
            tc.tile_pool(name="ps", bufs=1, space=bass.MemorySpace.PSUM) as ps,
        ):
            # Loads: tiny theta then butterfly data on the SP ring (spins up
            # ~1us earlier); masks for the corner path on the ACT ring.
            tht = cp.tile([2, 130], f32, tag="tht")
            nc.sync.dma_start(out=tht[:], in_=th[:])
            dt = cp.tile([128, DW], f32, tag="dt")
            nc.sync.dma_start(out=dt[:], in_=data[:])
            at = cp.tile([K, AW], f32, tag="at")
            nc.scalar.dma_start(out=at[:], in_=masks[:])

            # Replicate RAW theta to 128 partitions the moment it lands
            # (rank-1 PE matmul, no ACT dependency; pa2 absorbs the theta
            # lane), then one 128-partition Sin straight off PSUM gives
            # (s, c) = (sin, cos) with a single cross-engine hop.
            pa2 = ps.tile([2, 2], f32, tag="abs2")
            nc.tensor.matmul(pa2[:], tht[:, 0:2], tht[:, 0:2], start=True, stop=True)
            ptht = ps.tile([128, 2], f32, tag="ptht")
            nc.tensor.matmul(ptht[:], tht[:, 2:130], tht[:, 0:2], start=True, stop=True)
            acts = cp.tile([128, 2], f32, tag="acts")
            nc.scalar.activation(acts[:], ptht[:], Act.Sin)
            absq = cp.tile([2, 2], f32, tag="absq")
            nc.scalar.activation(absq[:], acts[0:2, 0:2], Act.Copy)
            q1 = wp.tile([128, HW], f32, tag="q1")
            nc.scalar.activation(q1[:], dt[:, 0:HW], Act.Copy, scale=acts[:, 0:1])

            # DVE staging copy absorbs the ACT tick for DVE.
            scd = cp.tile([128, 2], f32, tag="scd")
            nc.vector.tensor_copy(scd[:], acts[:])

            mk = cp.tile([K, 193], f32, tag="mk")
            nc.vector.tensor_copy(mk[:], at[:, 0:193])
            # PE absorber for the masks lane (so py2/py carry only the DVE wait)
            pa = ps.tile([K, K], f32, tag="abs")
            nc.tensor.matmul(pa[:], at[:, 0:K], at[:, 0:K], start=True, stop=True)
            # cos mask is packed TRANSPOSED (antisymmetric: cosm^T = -cosm),
            # so n = B^T = s*sinm + idm + c*cosm^T needs only two fused ops.
            tmp = cp.tile([K, K], f32, tag="tmp")
            nc.vector.scalar_tensor_tensor(tmp[:], mk[:, 64:128], scd[0:K, 0:1], mk[:, 128:192], Alu.mult, Alu.add)
            n = cp.tile([K, K], f32, tag="n")
            nc.vector.scalar_tensor_tensor(n[:], mk[:, 0:64], scd[0:K, 1:2], tmp[:], Alu.mult, Alu.add)

            # Corner on the PE: py2 first so r1's single PE wait (on the
            # later py) also covers it.
            py2 = ps.tile([K, K], f32, tag="y2")
            nc.tensor.matmul(py2[:], at[:, 257:321], n[:], start=True, stop=True)
            py = ps.tile([K, K], f32, tag="y")
            nc.tensor.matmul(py[:], at[:, 193:257], n[:], start=True, stop=True)

            # Odd plane entirely on DVE: q3 = c*E, outO = s*O - q3; the even
            # combine outE = c*O + q1 follows once ACT's q1 lands.
            q3 = wp.tile([128, HW], f32, tag="q3")
            nc.vector.tensor_scalar_mul(q3[:], dt[:, 0:HW], scd[:, 1:2])
            osbo = wp.tile([128, HW], f32, tag="osbo")
            nc.vector.scalar_tensor_tensor(osbo[:], dt[:, HW:DW], scd[:, 0:1], q3[:], Alu.mult, Alu.subtract)
            osbe = wp.tile([128, HW], f32, tag="osbe")
            nc.vector.scalar_tensor_tensor(osbe[:], dt[:, HW:DW], scd[:, 1:2], q1[:], Alu.mult, Alu.add)

            # Stores: odd plane on the SP ring, even plane on the ACT ring.
            nc.sync.dma_start(out=outs[:, HW:DW], in_=osbo[:])
            nc.scalar.dma_start(out=outs[:, 0:HW], in_=osbe[:])

            # Corner combine + store (ACT ring): outc = s*py + (+-c)*py2.
            ccv = cp.tile([K, 1], f32, tag="ccv")
            nc.vector.tensor_scalar_mul(ccv[:], mk[:, 192:193], scd[0:K, 1:2])
            r1 = wp.tile([K, K], f32, tag="r1")
            nc.vector.tensor_scalar_mul(r1[:], py[:], scd[0:K, 0:1])
            oc = wp.tile([K, K], f32, tag="oc")
            nc.vector.scalar_tensor_tensor(oc[:], py2[:], ccv[:, 0:1], r1[:], Alu.mult, Alu.add)
            nc.sync.dma_start(out=outc[:], in_=oc[:])

    return nc


def _get_nc():
    if "nc" not in _CACHE:
        _CACHE["nc"] = _build_nc()
    return _CACHE["nc"]


def _in_maps(input_state, angle, cos_matrix, sin_matrix, id_matrix):
    rho = np.ascontiguousarray(np.asarray(input_state, dtype=np.float32))
    assert rho.shape == (N_FULL, N_FULL)
    theta = np.float32(np.asarray(angle))

    corner = lambda m: np.asarray(m, dtype=np.float32)[0:K, 0:K]
    am = np.zeros((K, AW), dtype=np.float32)
    am[:, 0:64] = corner(cos_matrix).T
    am[:, 64:128] = corner(sin_matrix)
    am[:, 128:192] = corner(id_matrix)
    am[0:K:2, 192] = 1.0
    am[1:K:2, 192] = -1.0
    tht = np.zeros((2, 130), dtype=np.float32)
    tht[0, 0] = theta
    tht[0, 1] = theta + np.float32(np.pi / 2)
    tht[0, 2:130] = 1.0

    maps = []
    for c in range(N_CORES):
        a = am if c else am.copy()
        if c == 0:
            a[:, 193:257] = rho[0:K, 0:K]
            a[:, 257:321] = rho[0:K, 0:K].reshape(K, K // 2, 2)[:, :, ::-1].reshape(K, K)
        pos = np.zeros((NG * 128, K), dtype=np.float32)
        pos[0:RW] = rho[0:K, K + c * RW : K + (c + 1) * RW].T
        pos[RW : 2 * RW] = rho[K + c * RW : K + (c + 1) * RW, 0:K]
        # [1024, 64] -> per-group packing [128, NG*32] for even/odd planes
        ev = pos[:, 0::2].reshape(NG, 128, K // 2).transpose(1, 0, 2).reshape(128, HW)
        od = pos[:, 1::2].reshape(NG, 128, K // 2).transpose(1, 0, 2).reshape(128, HW)
        d = np.empty((128, DW), dtype=np.float32)
        d[:, 0:HW] = ev
        d[:, HW:DW] = od
        maps.append({"th": tht, "masks": a, "data": d})
    return maps


def _assemble(input_state, results):
    full = np.array(np.asarray(input_state, dtype=np.float32), copy=True)
    vals = np.empty((NG * 128, K), dtype=np.float32)
    for c in range(N_CORES):
        o = results[c]["outs"]
        vals[:, 0::2] = o[:, 0:HW].reshape(128, NG, K // 2).transpose(1, 0, 2).reshape(NG * 128, K // 2)
        vals[:, 1::2] = o[:, HW:DW].reshape(128, NG, K // 2).transpose(1, 0, 2).reshape(NG * 128, K // 2)
        full[0:K, K + c * RW : K + (c + 1) * RW] = vals[0:RW].T
        full[K + c * RW : K + (c + 1) * RW, 0:K] = vals[RW : 2 * RW]
    full[0:K, 0:K] = results[0]["outc"].T
    return full


def run(input_state, angle, cos_matrix, sin_matrix, id_matrix, **spmd_kwargs):
    from concourse.bass_utils import run_bass_kernel_spmd

    nc = _get_nc()
    maps = _in_maps(input_state, angle, cos_matrix, sin_matrix, id_matrix)
    res = run_bass_kernel_spmd(nc, maps, list(range(N_CORES)), **spmd_kwargs)
    return _assemble(input_state, res.results).astype(np.float32, copy=False), res


def kernel(input_state, angle, cos_matrix, sin_matrix, id_matrix):
    full, _ = run(input_state, angle, cos_matrix, sin_matrix, id_matrix)
    return full
